# revision 1
# baseline (speedup 1.0000x reference)
"""Trainium2 Bass kernel for nn_Net_74552042324489.

Data-parallel over batch n=8 across 8 NeuronCores (1 sample/core).
Per-core pipeline:
  cam = fc8_w @ _4            -> norm/suppress -> camT5 = [bg|fg supp|ones]^T
  x2r = bilinear(x2,112->56)   (stride-2 DVE trick, align_corners)
  f8_3 = relu(f83_w @ x2r)
  f8_4 = relu(f84_w @ deep3)
  x_s = bilinear(x,448->56)    (dense resize-matrix matmuls on PE)
  f = [f8_4; f8_3; x_s]        (channel-permuted; qk weights permuted to match)
  q,k = Wqk @ f
  Attention: S blocked [h=128p, k free]; exp on ScalarE (no max-sub needed:
  |S|<~30); second matmul lhsT=[camT|ones] fuses numerator + softmax denom;
  divide at the end.  Output [4, 3136] per core.
"""

import os
import sys

sys.path.insert(0, "/opt/trn_rl_repo")

from contextlib import ExitStack

import numpy as np

import concourse.bass as bass
import concourse.tile as tile
from concourse import bacc, mybir
from concourse.bass_utils import run_bass_kernel_spmd
from concourse.masks import make_identity

F32 = mybir.dt.float32
BF16 = mybir.dt.bfloat16
F32R = mybir.dt.float32r
AF = mybir.ActivationFunctionType
ALU = mybir.AluOpType

HW = 3136  # 56*56
N_CORES = 8

_CACHE = {}


def _resize_mat(h_in: int, h_out: int) -> np.ndarray:
    """Dense [h_in, h_out] bilinear align_corners=True resize matrix."""
    ys = np.linspace(0.0, h_in - 1.0, h_out).astype(np.float32)
    y0 = np.floor(ys).astype(np.int64)
    y1 = np.minimum(y0 + 1, h_in - 1)
    w = (ys - y0).astype(np.float32)
    R = np.zeros((h_in, h_out), np.float32)
    for i in range(h_out):
        R[y0[i], i] += 1.0 - w[i]
        R[y1[i], i] += w[i]
    return R


def _resize_coeffs_112() -> tuple[np.ndarray, np.ndarray]:
    """Per-output-col (0..54) weights for the stride-2 112->56 resize."""
    ys = np.linspace(0.0, 111.0, 56).astype(np.float32)
    y0 = np.floor(ys).astype(np.int64)
    w = (ys - y0).astype(np.float32)
    # structural property (verified): y0[i] == 2i for i < 55; y0[55] == 111
    a = (1.0 - w).astype(np.float32)  # weight of in[2i]
    b = w.astype(np.float32)          # weight of in[2i+1]
    return a, b


def _build_program():
    nc = bacc.Bacc(
        "TRN2", target_bir_lowering=False, debug=False, num_devices=N_CORES
    )

    # ---- DRAM I/O ----
    d_x4 = nc.dram_tensor("x4", [512, HW], F32, kind="ExternalInput")
    d_deep3 = nc.dram_tensor("deep3", [320, HW], F32, kind="ExternalInput")
    d_x2 = nc.dram_tensor("x2", [128, 112 * 112], F32, kind="ExternalInput")
    d_x = nc.dram_tensor("x", [3, 448, 448], BF16, kind="ExternalInput")
    d_fc8T = nc.dram_tensor("fc8T", [512, 4], F32, kind="ExternalInput")
    d_f83T = nc.dram_tensor("f83T", [128, 64], F32, kind="ExternalInput")
    d_f84T = nc.dram_tensor("f84T", [320, 128], F32, kind="ExternalInput")
    d_qkA = nc.dram_tensor("qkA", [128, 384], F32, kind="ExternalInput")
    d_qkB = nc.dram_tensor("qkB", [67, 384], F32, kind="ExternalInput")
    d_a112 = nc.dram_tensor("a112", [128, 56], F32, kind="ExternalInput")
    d_b112 = nc.dram_tensor("b112", [128, 56], F32, kind="ExternalInput")
    d_rh = nc.dram_tensor("rh448", [448, 56], BF16, kind="ExternalInput")
    d_rw = nc.dram_tensor("rw448", [448, 56], BF16, kind="ExternalInput")
    d_out = nc.dram_tensor("out", [4, HW], F32, kind="ExternalOutput")

    EPS = 1e-05

    with tile.TileContext(nc) as tc, ExitStack() as top:
        wpool = top.enter_context(tc.tile_pool(name="wpool", bufs=1))
        persist = top.enter_context(tc.tile_pool(name="persist", bufs=1))
        small = top.enter_context(tc.tile_pool(name="small", bufs=2))

        # ---- weights to SBUF (ordered by first use) ----
        a112 = wpool.tile([128, 56], F32, tag="a112")
        nc.sync.dma_start(a112[:], d_a112.ap())
        b112 = wpool.tile([128, 56], F32, tag="b112")
        nc.sync.dma_start(b112[:], d_b112.ap())
        fc8T = wpool.tile([128, 4, 4], F32, tag="fc8T")
        nc.sync.dma_start(fc8T[:], d_fc8T.ap().rearrange("(k p) o -> p k o", p=128))
        f84T_0 = wpool.tile([128, 128], F32, tag="f84T0")
        nc.sync.dma_start(f84T_0[:], d_f84T.ap()[0:128, :])
        f84T_1 = wpool.tile([128, 128], F32, tag="f84T1")
        nc.sync.dma_start(f84T_1[:], d_f84T.ap()[128:256, :])
        f84T_2 = wpool.tile([64, 128], F32, tag="f84T2")
        nc.sync.dma_start(f84T_2[:], d_f84T.ap()[256:320, :])
        f83T = wpool.tile([128, 64], F32, tag="f83T")
        nc.sync.dma_start(f83T[:], d_f83T.ap())
        rh = wpool.tile([112, 4, 56], BF16, tag="rh")
        nc.sync.dma_start(rh[:], d_rh.ap().rearrange("(k p) o -> p k o", p=112))
        rw = wpool.tile([112, 4, 56], BF16, tag="rw")
        nc.sync.dma_start(rw[:], d_rw.ap().rearrange("(k p) o -> p k o", p=112))
        qkA = wpool.tile([128, 384], F32, tag="qkA")
        nc.sync.dma_start(qkA[:], d_qkA.ap())
        qkB = wpool.tile([67, 384], F32, tag="qkB")
        nc.sync.dma_start(qkB[:], d_qkB.ap())
        ident = wpool.tile([128, 128], F32, tag="ident")
        make_identity(nc, ident[:])
        f84R_0 = wpool.tile([128, 128], F32R, tag="f84R0")
        nc.vector.tensor_copy(f84R_0[:], f84T_0[:])
        f84R_1 = wpool.tile([128, 128], F32R, tag="f84R1")
        nc.vector.tensor_copy(f84R_1[:], f84T_1[:])
        f84R_2 = wpool.tile([64, 128], F32R, tag="f84R2")
        nc.vector.tensor_copy(f84R_2[:], f84T_2[:])
        f83R = wpool.tile([128, 64], F32R, tag="f83R")
        nc.vector.tensor_copy(f83R[:], f83T[:])
        qkAR = wpool.tile([128, 384], F32R, tag="qkAR")
        nc.vector.tensor_copy(qkAR[:], qkA[:])
        qkBR = wpool.tile([67, 384], F32R, tag="qkBR")
        nc.vector.tensor_copy(qkBR[:], qkB[:])

        camT5 = persist.tile([128, 125], BF16, tag="camT5")  # 25 h-blocks x 5
        f_a = persist.tile([128, HW], F32R, tag="f_a")  # = f8_4
        f_b = persist.tile([67, HW], F32R, tag="f_b")  # = [f8_3(64); x_s(3)]
        qA = persist.tile([128, HW], BF16, tag="qA")
        qB = persist.tile([64, HW], BF16, tag="qB")
        kA = persist.tile([128, HW], BF16, tag="kA")
        kB = persist.tile([64, HW], BF16, tag="kB")
        out_sb = persist.tile([4, HW], F32, tag="out_sb")

        # h-block partition sizes: 24 x 128 + 1 x 64
        HBLK = [(i * 128, 128) for i in range(24)] + [(3072, 64)]
        # free-dim 512 chunks of 3136: 6 x 512 + 64
        NCH = [(i * 512, 512) for i in range(6)] + [(3072, 64)]

        # ================= P2: x2 -> x2r (stride-2 bilinear) =================
        # Emitted first so the long DVE resize chain overlaps the PE conv
        # phases (cam, f8_4) that only need DMA inputs.
        with tc.tile_pool(name="p2w", bufs=1) as p2w, \
             tc.tile_pool(name="p2s", bufs=2) as p2s, \
             tc.tile_pool(name="p2r", bufs=1) as p2r:
            x2w = p2w.tile([128, 112 * 56], F32, tag="x2w")  # after W-resize
            x2wv = x2w[:].rearrange("p (h w) -> p h w", h=112)
            HC = 14  # h rows per W-stage chunk
            for hc in range(112 // HC):
                st = p2s.tile([128, HC * 112], F32, tag="x2st")
                nc.sync.dma_start(
                    st[:], d_x2.ap()[:, hc * HC * 112:(hc + 1) * HC * 112]
                )
                sv = st[:].rearrange("p (h w) -> p h w", h=HC)
                dst = x2wv[:, hc * HC:(hc + 1) * HC, :]
                even = sv[:, :, 0:110:2]   # 55 taps
                odd = sv[:, :, 1:111:2]
                abc = a112[:, 0:55].unsqueeze(1).broadcast_to([128, HC, 55])
                bbc = b112[:, 0:55].unsqueeze(1).broadcast_to([128, HC, 55])
                t1 = p2s.tile([128, HC, 55], F32, tag="t1")
                nc.vector.tensor_tensor(t1[:], even, abc, op=ALU.mult)
                t2 = p2s.tile([128, HC, 55], F32, tag="t2")
                nc.vector.tensor_tensor(t2[:], odd, bbc, op=ALU.mult)
                nc.vector.tensor_tensor(dst[:, :, 0:55], t1[:], t2[:], op=ALU.add)
                nc.vector.tensor_copy(dst[:, :, 55:56], sv[:, :, 111:112])

            x2r = p2r.tile([128, HW], F32R, tag="x2r")
            x2rv = x2r[:].rearrange("p (h w) -> p h w", h=56)
            for jc, jl in ((0, 28), (28, 27)):
                everow = x2wv[:, 2 * jc:2 * (jc + jl) - 1:2, :]
                oddrow = x2wv[:, 2 * jc + 1:2 * (jc + jl):2, :]
                arow = a112[:, jc:jc + jl].unsqueeze(2).broadcast_to([128, jl, 56])
                brow = b112[:, jc:jc + jl].unsqueeze(2).broadcast_to([128, jl, 56])
                t3 = p2s.tile([128, 28, 56], F32, tag="t1")
                nc.vector.tensor_tensor(t3[:, 0:jl, :], everow, arow, op=ALU.mult)
                t4 = p2s.tile([128, 28, 56], F32, tag="t2")
                nc.vector.tensor_tensor(t4[:, 0:jl, :], oddrow, brow, op=ALU.mult)
                nc.vector.tensor_tensor(
                    x2rv[:, jc:jc + jl, :], t3[:, 0:jl, :], t4[:, 0:jl, :], op=ALU.add
                )
            nc.vector.tensor_copy(x2rv[:, 55:56, :], x2wv[:, 111:112, :])


            # ---- f8_4 = relu(f84R.T @ deep3) -> f_a (PE work under resize) ----
            with tc.tile_pool(name="p5s", bufs=4) as p5s, \
                 tc.tile_pool(name="p5p", bufs=4,
                              space=bass.MemorySpace.PSUM) as p5p:
                DCH = [(0, 128), (128, 128), (256, 64)]
                for no, nl in NCH:
                    fp = p5p.tile([128, 512], F32, tag="f4psum")
                    for ci, (co, cl) in enumerate(DCH):
                        st5 = p5s.tile([128, 512], F32, tag="d3st")
                        if no == 0:
                            for sl in range(cl // 32):
                                nc.sync.dma_start(
                                    st5[32 * sl:32 * (sl + 1), 0:nl],
                                    d_deep3.ap()[co + 32 * sl:co + 32 * (sl + 1),
                                                 no:no + nl],
                                )
                        else:
                            nc.sync.dma_start(
                                st5[0:cl, 0:nl],
                                d_deep3.ap()[co:co + cl, no:no + nl],
                            )
                        d3r = p5s.tile([128, 512], F32R, tag="d3r")
                        nc.scalar.copy(d3r[0:cl, 0:nl], st5[0:cl, 0:nl])
                        w = (f84R_0, f84R_1, f84R_2)[ci]
                        nc.tensor.matmul(
                            fp[:, 0:nl], w[:], d3r[0:cl, 0:nl],
                            start=(ci == 0), stop=(ci == 2),
                        )
                    nc.scalar.activation(f_a[:, no:no + nl], fp[:, 0:nl], AF.Relu)

            # ================= P4: x -> x_s -> f_b[64:67] =================
            with tc.tile_pool(name="p4s", bufs=2) as p4s, \
                 tc.tile_pool(name="p4sb", bufs=1) as p4sb, \
                 tc.tile_pool(name="p4p", bufs=1, space=bass.MemorySpace.PSUM) as p4p:
                xh = p4sb.tile([56, 3, 448], BF16, tag="xh")
                xps = [
                    p4p.tile([56, 448], F32, tag=f"xhp{c}", name=f"xhp{c}")
                    for c in range(3)
                ]
                xdr = d_x.ap().rearrange("c h w -> h c w")
                for hc in range(4):
                    st = p4s.tile([112, 3, 448], BF16, tag="xst")
                    nc.sync.dma_start(st[:], xdr[112 * hc:112 * (hc + 1), :, :])
                    for c in range(3):
                        nc.tensor.matmul(
                            xps[c][:], rh[:, hc, :], st[:, c, :],
                            start=(hc == 0), stop=(hc == 3),
                        )
                for c in range(3):
                    nc.vector.tensor_copy(xh[:, c, :], xps[c][:])

                xhT = p4sb.tile([112, 12, 56], BF16, tag="xhT")
                idb = p4sb.tile([128, 128], BF16, tag="idb")
                nc.vector.tensor_copy(idb[:], ident[:])
                for c in range(3):
                    for wc in range(4):
                        tp = p4p.tile([112, 56], BF16, tag="xtp", bufs=2)
                        nc.tensor.transpose(
                            tp[:], xh[:, c, 112 * wc:112 * (wc + 1)], idb[0:56, 0:56]
                        )
                        nc.vector.tensor_copy(xhT[:, c * 4 + wc, :], tp[:])
                xs3 = p4sb.tile([3, HW], F32, tag="xs3")
                for c in range(3):
                    wp = p4p.tile([56, 56], F32, tag="xwp", bufs=2)
                    for wc in range(4):
                        nc.tensor.matmul(
                            wp[:], xhT[:, c * 4 + wc, :], rw[:, wc, :],
                            start=(wc == 0), stop=(wc == 3),
                        )
                    ws = p4s.tile([56, 56], F32, tag="xws")
                    nc.vector.tensor_copy(ws[:], wp[:])
                    nc.sync.dma_start(xs3[c:c + 1, :], ws[:])
                nc.vector.tensor_copy(f_b[64:67, :], xs3[:])


            # ---- cam = fc8T.T @ _4 (x4 streamed last; cam needed only at P7) ----
            with tc.tile_pool(name="p1s", bufs=4) as p1s, \
                 tc.tile_pool(name="p1p", bufs=2, space=bass.MemorySpace.PSUM) as p1p, \
                 tc.tile_pool(name="p1sb", bufs=1) as p1sb:
                cam = p1sb.tile([4, HW], F32, tag="cam")
                for no, nl in NCH:
                    cp = p1p.tile([4, 512], F32, tag="campsum")
                    for ck in range(4):
                        st = p1s.tile([128, 512], F32, tag="x4st")
                        nc.sync.dma_start(
                            st[:, 0:nl],
                            d_x4.ap()[128 * ck:128 * (ck + 1), no:no + nl],
                        )
                        nc.tensor.matmul(
                            cp[:, 0:nl], fc8T[:, ck, :], st[:, 0:nl],
                            start=(ck == 0), stop=(ck == 3),
                        )
                    nc.scalar.copy(cam[:, no:no + nl], cp[:, 0:nl])

                # ---- P3: f8_3 = relu(f83T.T @ x2r) -> f_b[0:64] ----
                with tc.tile_pool(name="p3p", bufs=2,
                                  space=bass.MemorySpace.PSUM) as p3p:
                    for no, nl in NCH:
                        fp3 = p3p.tile([64, 512], F32, tag="f3psum")
                        nc.tensor.matmul(
                            fp3[:, 0:nl], f83R[:], x2r[:, no:no + nl],
                            start=True, stop=True,
                        )
                        nc.scalar.activation(
                            f_b[0:64, no:no + nl], fp3[:, 0:nl], AF.Relu
                        )

                # ---- P1b: normalize, transpose, fg-suppress -> camT5 ----
                mn = small.tile([4, 1], F32, tag="mn")
                mx = small.tile([4, 1], F32, tag="mx")
                nc.vector.tensor_reduce(
                    mn[:], cam[:], axis=mybir.AxisListType.X, op=ALU.min
                )
                nc.vector.tensor_reduce(
                    mx[:], cam[:], axis=mybir.AxisListType.X, op=ALU.max
                )
                rng = small.tile([4, 1], F32, tag="rng")
                nc.vector.tensor_tensor(rng[:], mx[:], mn[:], op=ALU.subtract)
                nc.vector.tensor_scalar_add(rng[:], rng[:], EPS)
                rs = small.tile([4, 1], F32, tag="rs")
                nc.vector.reciprocal(rs[:], rng[:])
                norm = p1sb.tile([4, HW], F32, tag="norm")
                nc.vector.tensor_scalar(
                    norm[:], cam[:], mn[:], rs[:], op0=ALU.subtract, op1=ALU.mult
                )

                camTall = p1sb.tile([128, 25, 4], F32, tag="camTall")
                nc.vector.memset(camTall[64:128, 24, :], 0.0)
                for bi, (ho, hl) in enumerate(HBLK):
                    tp = p1p.tile([128, 4], F32, tag="tpsum")
                    nc.tensor.transpose(
                        tp[0:hl, :], norm[:, ho:ho + hl], ident[0:4, 0:4]
                    )
                    nc.vector.tensor_copy(camTall[0:hl, bi, :], tp[0:hl, :])
                # vectorized over all 25 blocks at once
                c5v = camT5[:].rearrange("p (b f) -> p b f", f=5)
                nc.vector.memset(c5v[:, :, 4], 1.0)
                fm = p1sb.tile([128, 25], F32, tag="fm")
                nc.vector.tensor_reduce(
                    fm[:], camTall[:, :, 1:4], axis=mybir.AxisListType.X, op=ALU.max
                )
                nc.vector.tensor_scalar(
                    c5v[:, :, 0], fm[:], -1.0, 1.0, op0=ALU.mult, op1=ALU.add
                )
                msk = p1sb.tile([128, 25, 3], F32, tag="msk")
                fmb = fm[:].unsqueeze(2).broadcast_to([128, 25, 3])
                nc.vector.tensor_tensor(
                    msk[:], camTall[:, :, 1:4], fmb, op=ALU.is_ge
                )
                nc.vector.tensor_tensor(
                    c5v[:, :, 1:4], camTall[:, :, 1:4], msk[:], op=ALU.mult
                )

        # ================= P6: q, k =================
        with tc.tile_pool(name="p6p", bufs=4, space=bass.MemorySpace.PSUM) as p6p:
            MCH = [(qA, 0, 128), (qB, 128, 64), (kA, 192, 128), (kB, 320, 64)]
            for dst, mo, ml in MCH:
                for no, nl in NCH:
                    qp = p6p.tile([128, 512], F32, tag="qkpsum")
                    nc.tensor.matmul(
                        qp[0:ml, 0:nl], qkAR[:, mo:mo + ml], f_a[:, no:no + nl],
                        start=True, stop=False,
                    )
                    nc.tensor.matmul(
                        qp[0:ml, 0:nl], qkBR[:, mo:mo + ml], f_b[:, no:no + nl],
                        start=False, stop=True,
                    )
                    nc.vector.tensor_copy(dst[0:ml, no:no + nl], qp[0:ml, 0:nl])

        # ================= P7: attention =================
        with tc.tile_pool(name="p7e", bufs=6) as p7e, \
             tc.tile_pool(name="p7r", bufs=2) as p7r, \
             tc.tile_pool(name="p7s", bufs=2, space=bass.MemorySpace.PSUM) as p7s, \
             tc.tile_pool(name="p7o", bufs=2, space=bass.MemorySpace.PSUM) as p7o:
            # k-superblocks: 3 x 1024 + 1 x 64
            KSUP = [(0, 1024), (1024, 1024), (2048, 1024), (3072, 64)]
            for ko, kl in KSUP:
                nkb = (kl + 511) // 512
                pout = p7o.tile([5, 1024], F32, tag="pout")
                for bi, (ho, hl) in enumerate(HBLK):
                    sp = p7s.tile([128, 1024], F32, tag="spsum")
                    for kb in range(nkb):
                        kbl = min(512, kl - kb * 512)
                        nc.tensor.matmul(
                            sp[0:hl, kb * 512:kb * 512 + kbl], qA[:, ho:ho + hl],
                            kA[:, ko + kb * 512:ko + kb * 512 + kbl],
                            start=True, stop=False,
                        )
                    for kb in range(nkb):
                        kbl = min(512, kl - kb * 512)
                        nc.tensor.matmul(
                            sp[0:hl, kb * 512:kb * 512 + kbl], qB[:, ho:ho + hl],
                            kB[:, ko + kb * 512:ko + kb * 512 + kbl],
                            start=False, stop=True,
                        )
                    et = p7e.tile([128, 1024], BF16, tag="exptile")
                    nc.scalar.activation(et[0:hl, 0:kl], sp[0:hl, 0:kl], AF.Exp)
                    for kb in range(nkb):
                        kbl = min(512, kl - kb * 512)
                        nc.tensor.matmul(
                            pout[:, kb * 512:kb * 512 + kbl],
                            camT5[0:hl, bi * 5:bi * 5 + 5],
                            et[0:hl, kb * 512:kb * 512 + kbl],
                            start=(bi == 0), stop=(bi == 24),
                        )
                ot5 = p7r.tile([5, 1024], F32, tag="ot5")
                nc.vector.tensor_copy(ot5[:, 0:kl], pout[:, 0:kl])
                den = p7r.tile([1, 1024], F32, tag="den")
                nc.sync.dma_start(den[0:1, 0:kl], ot5[4:5, 0:kl])
                rcp = p7r.tile([1, 1024], F32, tag="rcp")
                rsc = p7r.tile([1, 1024], F32, tag="rsc")
                nc.vector.reciprocal_approx_accurate(
                    rcp[0:1, 0:kl], den[0:1, 0:kl], rsc[0:1, 0:kl]
                )
                rb = p7r.tile([4, 1024], F32, tag="rb")
                nc.gpsimd.partition_broadcast(rb[:, 0:kl], rcp[0:1, 0:kl])
                nc.gpsimd.tensor_tensor(
                    out_sb[:, ko:ko + kl], ot5[0:4, 0:kl], rb[:, 0:kl], op=ALU.mult
                )
                nc.sync.dma_start(
                    d_out.ap()[:, ko:ko + kl], out_sb[:, ko:ko + kl]
                )

    nc.compile()
    return nc


def _get_program():
    if "nc" not in _CACHE:
        _CACHE["nc"] = _build_program()
    return _CACHE["nc"]


def _host_prep(inputs: dict) -> list[dict]:
    x = np.ascontiguousarray(np.asarray(inputs["x"], np.float32))
    x2 = np.ascontiguousarray(np.asarray(inputs["x2"], np.float32))
    deep3 = np.ascontiguousarray(np.asarray(inputs["deep3"], np.float32))
    _4 = np.ascontiguousarray(np.asarray(inputs["_4"], np.float32))
    fc8_w = np.asarray(inputs["fc8_w"], np.float32)
    f83_w = np.asarray(inputs["f83_w"], np.float32)
    f84_w = np.asarray(inputs["f84_w"], np.float32)
    f91_w = np.asarray(inputs["f91_w"], np.float32)
    f92_w = np.asarray(inputs["f92_w"], np.float32)

    n = x.shape[0]
    fc8T = np.ascontiguousarray(fc8_w.T)  # [512, 4]
    f83T = np.ascontiguousarray(f83_w.T)  # [128, 64]
    f84T = np.ascontiguousarray(f84_w.T)  # [320, 128]
    # f channel permutation: [f8_4 (128), f8_3 (64), x_s (3)]
    perm = np.concatenate([np.arange(67, 195), np.arange(3, 67), np.arange(3)])
    wqk = np.concatenate([f91_w, f92_w], axis=0)[:, perm]  # [384, 195]
    wqkT = np.ascontiguousarray(wqk.T)  # [195, 384]
    qkA = np.ascontiguousarray(wqkT[0:128])
    qkB = np.ascontiguousarray(wqkT[128:195])
    a112, b112 = _resize_coeffs_112()
    import ml_dtypes

    BFNP = ml_dtypes.bfloat16
    a112 = np.ascontiguousarray(np.broadcast_to(a112, (128, 56)))
    b112 = np.ascontiguousarray(np.broadcast_to(b112, (128, 56)))
    rh448 = _resize_mat(448, 56).astype(BFNP)
    rw448 = rh448  # same matrix for H and W (448x448 -> 56x56)
    x = x.astype(BFNP)

    shared = {
        "fc8T": fc8T, "f83T": f83T, "f84T": f84T, "qkA": qkA, "qkB": qkB,
        "a112": a112, "b112": b112, "rh448": rh448, "rw448": rw448,
    }
    in_maps = []
    for i in range(n):
        m = dict(shared)
        m["x4"] = _4[i].reshape(512, HW)
        m["deep3"] = deep3[i].reshape(320, HW)
        m["x2"] = x2[i].reshape(128, 112 * 112)
        m["x"] = x[i]
        in_maps.append(m)
    return in_maps


def _install_ntff_hook() -> bool:
    """Register the NTFF profile hook that the agent image's antenv lacks."""
    try:
        import types

        import antenv

        if "antenv.axon_hooks" not in sys.modules:
            mod = types.ModuleType("antenv.axon_hooks")
            store = {"h": None}
            mod.set_axon_ntff_profile_hook = lambda h: store.update(h=h)
            mod.get_axon_ntff_profile_hook = lambda: store["h"]
            sys.modules["antenv.axon_hooks"] = mod
            antenv.axon_hooks = mod
            from trn_agent_boot.trn_boot import _ntff_profile_via_ctypes

            hook = _ntff_profile_via_ctypes("/opt/axon/libaxon_pjrt.so")
            if hook is None:
                return False
            mod.set_axon_ntff_profile_hook(hook)
        return sys.modules["antenv.axon_hooks"].get_axon_ntff_profile_hook() is not None
    except Exception as e:  # profiling is best-effort
        print(f"ntff hook install failed: {e}", file=sys.stderr)
        return False


def kernel(**inputs) -> np.ndarray:
    nc = _get_program()
    in_maps = _host_prep(inputs)
    trace = bool(int(os.environ.get("KERNEL_PROFILE", "0")))
    if trace:
        trace = _install_ntff_hook()
    res = run_bass_kernel_spmd(nc, in_maps, core_ids=list(range(N_CORES)),
                               trace=trace)
    _CACHE["last_result"] = res
    out = np.stack([r["out"] for r in res.results]).reshape(8, 4, 56, 56)
    return out.astype(np.float32)



# revision 3
# speedup vs baseline: 1.3435x; 1.3435x over previous
"""Trainium2 Bass kernel for nn_Net_74552042324489.

Data-parallel over batch n=8 across 8 NeuronCores (1 sample/core).
Per-core pipeline (v2):
  cam = fc8_w @ _4 in true fp32 (argmax-suppression is discontinuous ->
        needs fp32); cam h-block transposes stream during the matmul,
        min/max partials per chunk, normalization applied after the
        transpose via partition-broadcast scalars.
  f8_3 path: y2 = f83_w @ x2 on PE FIRST (channel reduce commutes with
        the bilinear resize), packed [128p = 64ch x 2 h-halves, 56 x 112],
        then a 3-op DVE separable resize with one combined host-built
        coefficient tile, then relu on ScalarE.
  f8_4 = relu(f84_w @ deep3) with deep3 resident bf16.
  x_s  = dense resize-matrix matmuls on PE.
  q,k  = Wqk @ f (bf16); h padded to 3200 so every h-block is 128 wide.
  Attention: S blocked [128h x 1024k] bf16; exp on ScalarE (no max-sub:
        |S|<~30); second matmul lhsT=[cam rows|ones|0-pad to 65] fuses
        numerator + softmax denominator; divide on DVE/GpSimd.
Stationary free dims and contractions padded (65 / 128) so the PE stays
in one tiling mode through the hot loops.  Inputs deep3/x2/x are bf16
(the smooth softmax path tolerates it), _4 stays fp32.
"""

import os
import sys

sys.path.insert(0, "/opt/trn_rl_repo")

from contextlib import ExitStack

import numpy as np

import concourse.bass as bass
import concourse.tile as tile
from concourse import bacc, mybir
from concourse.bass_utils import run_bass_kernel_spmd
from concourse.masks import make_identity

F32 = mybir.dt.float32
BF16 = mybir.dt.bfloat16
AF = mybir.ActivationFunctionType
ALU = mybir.AluOpType

HW = 3136       # 56*56
HWP = 3200      # h padded to 25*128
N_CORES = 8
EPS = 1e-05

_CACHE = {}


def _resize_mat(h_in: int, h_out: int) -> np.ndarray:
    """Dense [h_in, h_out] bilinear align_corners=True resize matrix."""
    ys = np.linspace(0.0, h_in - 1.0, h_out).astype(np.float32)
    y0 = np.floor(ys).astype(np.int64)
    y1 = np.minimum(y0 + 1, h_in - 1)
    w = (ys - y0).astype(np.float32)
    R = np.zeros((h_in, h_out), np.float32)
    for i in range(h_out):
        R[y0[i], i] += 1.0 - w[i]
        R[y1[i], i] += w[i]
    return R


def _resize_coeffs_112() -> tuple[np.ndarray, np.ndarray]:
    """112->56 align-corners taps: out[i] = a[i]*in[2i] + b[i]*in[2i+1];
    a[55]=0, b[55]=1 selects in[111]."""
    ys = np.linspace(0.0, 111.0, 56).astype(np.float64)
    y0 = np.floor(ys).astype(np.int64)
    w = ys - y0
    a = 1.0 - w
    b = w
    # structural: y0[i] == 2i for i < 55; y0[55] == 111 -> pair (110, 111)
    a[55], b[55] = 0.0, 1.0
    return a, b


def _combined_resize_coeff() -> np.ndarray:
    """C [128, 56, 112]: C[p, r, w'] = rowc[p, r] * colc[w'] such that
    resize = rowpair-sum(wpair-sum(y2 * C)).  Partitions 0-63 carry h rows
    0-55 (out rows 0-27); partitions 64-127 carry h rows 56-111 (out rows
    28-55)."""
    a, b = _resize_coeffs_112()
    colc = np.zeros(112, np.float64)
    colc[0::2] = a
    colc[1::2] = b
    rowc = np.zeros((128, 56), np.float64)
    for half in range(2):
        for lr in range(56):
            j = 28 * half + lr // 2
            rowc[64 * half:64 * half + 64, lr] = a[j] if lr % 2 == 0 else b[j]
    C = rowc[:, :, None] * colc[None, None, :]
    return C.astype(np.float32)


def _build_program():
    nc = bacc.Bacc(
        "TRN2", target_bir_lowering=False, debug=False, num_devices=N_CORES
    )

    # ---- DRAM I/O ----
    d_x2a = nc.dram_tensor("x2a", [128, 6272], BF16, kind="ExternalInput")
    d_x2b = nc.dram_tensor("x2b", [128, 6272], BF16, kind="ExternalInput")
    d_x4 = nc.dram_tensor("x4", [128, 4, HW], F32, kind="ExternalInput")
    d_deep3 = nc.dram_tensor("deep3", [384, HW], BF16, kind="ExternalInput")
    d_x = nc.dram_tensor("x", [448, 3, 448], BF16, kind="ExternalInput")
    d_fc8T = nc.dram_tensor("fc8T", [512, 65], F32, kind="ExternalInput")
    d_f83T = nc.dram_tensor("f83T", [128, 64], BF16, kind="ExternalInput")
    d_f84T = nc.dram_tensor("f84T", [384, 128], BF16, kind="ExternalInput")
    d_qkA = nc.dram_tensor("qkA", [128, 385], BF16, kind="ExternalInput")
    d_qkB = nc.dram_tensor("qkB", [67, 385], BF16, kind="ExternalInput")
    d_cres = nc.dram_tensor("cres", [128, 56 * 112], BF16, kind="ExternalInput")
    d_rh = nc.dram_tensor("rh448", [448, 56], BF16, kind="ExternalInput")
    d_rw = nc.dram_tensor("rw448", [448, 56], BF16, kind="ExternalInput")
    d_out = nc.dram_tensor("out", [4, HW], F32, kind="ExternalOutput")

    # free-dim chunking
    NCH = [(i * 512, 512) for i in range(6)] + [(3072, 64)]      # 3136
    NCHP = [(i * 512, 512) for i in range(6)] + [(3072, 128)]    # 3200
    NCH2 = [(i * 512, 512) for i in range(12)] + [(6144, 128)]   # 6272
    KSUP = [(0, 1024), (1024, 1024), (2048, 1024), (3072, 64)]

    with tile.TileContext(nc) as tc, ExitStack() as top:
        wpool = top.enter_context(tc.tile_pool(name="wpool", bufs=1))
        xin = top.enter_context(tc.tile_pool(name="xin", bufs=1))
        persist = top.enter_context(tc.tile_pool(name="persist", bufs=1))
        small = top.enter_context(tc.tile_pool(name="small", bufs=2))

        # ---- weights to SBUF ----
        f83T = wpool.tile([128, 64], BF16, tag="f83T")
        nc.sync.dma_start(f83T[:], d_f83T.ap())
        fc8T = wpool.tile([128, 4, 65], F32, tag="fc8T")
        nc.sync.dma_start(fc8T[:], d_fc8T.ap().rearrange("(k p) o -> p k o", p=128))
        cres = wpool.tile([128, 56, 112], BF16, tag="cres")
        nc.sync.dma_start(cres[:], d_cres.ap().rearrange("p (r w) -> p r w", r=56))
        f84T_0 = wpool.tile([128, 128], BF16, tag="f84T0")
        nc.sync.dma_start(f84T_0[:], d_f84T.ap()[0:128, :])
        f84T_1 = wpool.tile([128, 128], BF16, tag="f84T1")
        nc.sync.dma_start(f84T_1[:], d_f84T.ap()[128:256, :])
        f84T_2 = wpool.tile([128, 128], BF16, tag="f84T2")
        nc.sync.dma_start(f84T_2[:], d_f84T.ap()[256:384, :])
        rh = wpool.tile([112, 4, 56], BF16, tag="rh")
        nc.sync.dma_start(rh[:], d_rh.ap().rearrange("(k p) o -> p k o", p=112))
        rw = wpool.tile([112, 4, 56], BF16, tag="rw")
        nc.sync.dma_start(rw[:], d_rw.ap().rearrange("(k p) o -> p k o", p=112))
        qkA = wpool.tile([128, 385], BF16, tag="qkA")
        nc.sync.dma_start(qkA[:], d_qkA.ap())
        qkB = wpool.tile([67, 385], BF16, tag="qkB")
        nc.sync.dma_start(qkB[:], d_qkB.ap())
        ident = wpool.tile([128, 128], F32, tag="ident")
        make_identity(nc, ident[:])

        # persistent SBUF
        camTall = persist.tile([128, 25, 4], F32, tag="camTall")
        camT = persist.tile([128, 25, 65], BF16, tag="camT")
        f_a = persist.tile([128, HWP], BF16, tag="f_a")
        f_b = persist.tile([67, HWP], BF16, tag="f_b")
        qA = persist.tile([128, HWP], BF16, tag="qA")
        qB = persist.tile([65, HWP], BF16, tag="qB")
        kA = persist.tile([128, HWP], BF16, tag="kA")
        kB = persist.tile([65, HWP], BF16, tag="kB")
        xsb = persist.tile([112, 4, 3, 448], BF16, tag="xsb")
        out_sb = persist.tile([4, HW], F32, tag="out_sb")

        # zero pads once
        nc.vector.memset(camT[:], 0.0)
        nc.vector.memset(f_a[:, HW:HWP], 0.0)
        nc.vector.memset(f_b[:, HW:HWP], 0.0)
        nc.vector.memset(qB[64:65, :], 0.0)
        nc.vector.memset(kB[64:65, :], 0.0)
        nc.vector.memset(camTall[64:128, 24, :], 0.0)

        mn = small.tile([4, 1], F32, tag="mn")
        mx = small.tile([4, 1], F32, tag="mx")

        # ============ phase A: y2 = f83 @ x2 ; cam ; f8_4 ; resize ==========
        with tc.tile_pool(name="inA", bufs=1) as inA, \
             tc.tile_pool(name="inA4", bufs=3) as inA4, \
             tc.tile_pool(name="sbA", bufs=1) as sbA, \
             tc.tile_pool(name="pAy", bufs=2, space=bass.MemorySpace.PSUM) as pAy, \
             tc.tile_pool(name="pAc", bufs=2, space=bass.MemorySpace.PSUM) as pAc, \
             tc.tile_pool(name="pAf", bufs=2, space=bass.MemorySpace.PSUM) as pAf:
            # input DMAs (emitted first = high priority)
            x2h0 = inA.tile([128, 6272], BF16, tag="x2h0")
            nc.sync.dma_start(x2h0[:], d_x2a.ap())
            x2h1 = inA.tile([128, 6272], BF16, tag="x2h1")
            nc.sync.dma_start(x2h1[:], d_x2b.ap())
            nc.sync.dma_start(
                xsb[:], d_x.ap().rearrange("(hc p) c w -> p hc c w", p=112)
            )
            d3_0 = inA.tile([128, HW], BF16, tag="d3_0")
            nc.sync.dma_start(d3_0[:], d_deep3.ap()[0:128, :])
            d3_1 = inA.tile([128, HW], BF16, tag="d3_1")
            nc.sync.dma_start(d3_1[:], d_deep3.ap()[128:256, :])
            d3_2 = inA.tile([128, HW], BF16, tag="d3_2")
            nc.sync.dma_start(d3_2[:], d_deep3.ap()[256:384, :])

            y2 = sbA.tile([128, 56, 112], BF16, tag="y2")
            cam = sbA.tile([4, HW], F32, tag="cam")

            # ---- y2 = f83 @ x2, packed halves ----
            y2v = y2[:].rearrange("p r w -> p (r w)")
            for no, nl in NCH2:
                pp = pAy.tile([128, 512], F32, tag="y2p")
                nc.tensor.matmul(
                    pp[0:64, 0:nl], f83T[:], x2h0[:, no:no + nl],
                    start=True, stop=True,
                )
                nc.tensor.matmul(
                    pp[64:128, 0:nl], f83T[:], x2h1[:, no:no + nl],
                    start=True, stop=True,
                )
                nc.vector.tensor_copy(y2v[:, no:no + nl], pp[:, 0:nl])

            # ---- cam (true fp32) + streamed transposes + minmax partials ----
            for ci, (no, nl) in enumerate(NCH):
                x4t = inA4.tile([128, 4, 512], F32, tag="x4c")
                nc.sync.dma_start(x4t[:, :, 0:nl], d_x4.ap()[:, :, no:no + nl])
                cp = pAc.tile([65, 512], F32, tag="campsum")
                for ck in range(4):
                    nc.tensor.matmul(
                        cp[:, 0:nl], fc8T[:, ck, :], x4t[:, ck, 0:nl],
                        start=(ck == 0), stop=(ck == 3),
                    )
                nc.scalar.copy(cam[:, no:no + nl], cp[0:4, 0:nl])
                pmn = small.tile([4, 1], F32, tag="pmn")
                nc.vector.tensor_reduce(
                    pmn[:], cp[0:4, 0:nl], axis=mybir.AxisListType.X, op=ALU.min
                )
                pmx = small.tile([4, 1], F32, tag="pmx")
                nc.vector.tensor_reduce(
                    pmx[:], cp[0:4, 0:nl], axis=mybir.AxisListType.X, op=ALU.max
                )
                if ci == 0:
                    nc.vector.tensor_copy(mn[:], pmn[:])
                    nc.vector.tensor_copy(mx[:], pmx[:])
                else:
                    nc.vector.tensor_tensor(mn[:], mn[:], pmn[:], op=ALU.min)
                    nc.vector.tensor_tensor(mx[:], mx[:], pmx[:], op=ALU.max)
                # stream transposes of this chunk's h-blocks
                for bi in range(no // 128, (no + nl + 127) // 128):
                    hl = min(128, HW - 128 * bi)
                    tp = pAc.tile([128, 4], F32, tag="tpsum")
                    nc.tensor.transpose(
                        tp[0:hl, :], cam[:, 128 * bi:128 * bi + hl],
                        ident[0:4, 0:4],
                    )
                    nc.vector.tensor_copy(camTall[0:hl, bi, :], tp[0:hl, :])

            # ---- f8_4 = relu(f84T.T @ deep3) -> f_a ----
            D3 = ((d3_0, f84T_0), (d3_1, f84T_1), (d3_2, f84T_2))
            for no, nl in NCH:
                fp = pAf.tile([128, 512], F32, tag="f4psum")
                for ci2, (dt_, wt_) in enumerate(D3):
                    nc.tensor.matmul(
                        fp[:, 0:nl], wt_[:], dt_[:, no:no + nl],
                        start=(ci2 == 0), stop=(ci2 == 2),
                    )
                nc.scalar.activation(f_a[:, no:no + nl], fp[:, 0:nl], AF.Relu)

            # ---- resize y2 -> f8_3 packed, on DVE ----
            T = sbA.tile([128, 56, 112], BF16, tag="rt")
            nc.vector.tensor_tensor(T[:], y2[:], cres[:], op=ALU.mult)
            U = sbA.tile([128, 56, 56], BF16, tag="ru")
            nc.vector.tensor_tensor(
                U[:], T[:, :, 0:111:2], T[:, :, 1:112:2], op=ALU.add
            )
            R = sbA.tile([128, 28, 56], BF16, tag="rr")
            nc.vector.tensor_tensor(
                R[:], U[:, 0:55:2, :], U[:, 1:56:2, :], op=ALU.add
            )
            f83p = sbA.tile([128, 28 * 56], BF16, tag="f83p")
            nc.scalar.activation(
                f83p[:], R[:].rearrange("p r w -> p (r w)"), AF.Relu
            )
            nc.sync.dma_start(f_b[0:64, 0:1568], f83p[0:64, :])
            nc.sync.dma_start(f_b[0:64, 1568:3136], f83p[64:128, :])

        # ============ phase B: x_s ; camT5 finalize ============
        with tc.tile_pool(name="pBs", bufs=2) as pBs, \
             tc.tile_pool(name="pBsb", bufs=1) as pBsb, \
             tc.tile_pool(name="pBp", bufs=1, space=bass.MemorySpace.PSUM) as pBp, \
             tc.tile_pool(name="pBt", bufs=2, space=bass.MemorySpace.PSUM) as pBt:
            xh = pBsb.tile([56, 3, 448], BF16, tag="xh")
            xps = [
                pBp.tile([56, 448], F32, tag=f"xhp{c}", name=f"xhp{c}")
                for c in range(3)
            ]
            for hc in range(4):
                for c in range(3):
                    nc.tensor.matmul(
                        xps[c][:], rh[:, hc, :], xsb[:, hc, c, :],
                        start=(hc == 0), stop=(hc == 3),
                    )
            for c in range(3):
                nc.vector.tensor_copy(xh[:, c, :], xps[c][:])

            xhT = pBsb.tile([112, 12, 56], BF16, tag="xhT")
            idb = pBsb.tile([128, 128], BF16, tag="idb")
            nc.vector.tensor_copy(idb[:], ident[:])
            for c in range(3):
                for wc in range(4):
                    tp2 = pBt.tile([112, 56], BF16, tag="xtp")
                    nc.tensor.transpose(
                        tp2[:], xh[:, c, 112 * wc:112 * (wc + 1)], idb[0:56, 0:56]
                    )
                    nc.vector.tensor_copy(xhT[:, c * 4 + wc, :], tp2[:])
            for c in range(3):
                wp = pBt.tile([56, 56], F32, tag="xwp")
                for wc in range(4):
                    nc.tensor.matmul(
                        wp[:], xhT[:, c * 4 + wc, :], rw[:, wc, :],
                        start=(wc == 0), stop=(wc == 3),
                    )
                ws = pBs.tile([56, 56], BF16, tag="xws")
                nc.vector.tensor_copy(ws[:], wp[:])
                nc.sync.dma_start(f_b[64 + c:65 + c, 0:HW], ws[:])

            # ---- camT5 finalize: normalize on transposed layout ----
            rng = small.tile([4, 1], F32, tag="rng")
            nc.vector.tensor_tensor(rng[:], mx[:], mn[:], op=ALU.subtract)
            nc.vector.tensor_scalar_add(rng[:], rng[:], EPS)
            rs = small.tile([4, 1], F32, tag="rs")
            nc.vector.reciprocal(rs[:], rng[:])
            mrow = small.tile([1, 8], F32, tag="mrow")
            nc.sync.dma_start(mrow[0:1, 0:4], mn[:])
            nc.sync.dma_start(mrow[0:1, 4:8], rs[:])
            mbc = pBsb.tile([128, 8], F32, tag="mbc")
            nc.gpsimd.partition_broadcast(mbc[:], mrow[0:1, :])
            normT = pBsb.tile([128, 25, 4], F32, tag="normT")
            nc.vector.tensor_tensor(
                normT[:], camTall[:],
                mbc[:, 0:4].unsqueeze(1).broadcast_to([128, 25, 4]),
                op=ALU.subtract,
            )
            nc.vector.tensor_tensor(
                normT[:], normT[:],
                mbc[:, 4:8].unsqueeze(1).broadcast_to([128, 25, 4]),
                op=ALU.mult,
            )
            c5v = camT[:]  # [128, 25, 65]
            nc.vector.memset(c5v[:, :, 4], 1.0)
            fm = pBsb.tile([128, 25], F32, tag="fm")
            nc.vector.tensor_reduce(
                fm[:], normT[:, :, 1:4], axis=mybir.AxisListType.X, op=ALU.max
            )
            nc.vector.tensor_scalar(
                c5v[:, :, 0], fm[:], -1.0, 1.0, op0=ALU.mult, op1=ALU.add
            )
            msk = pBsb.tile([128, 25, 3], F32, tag="msk")
            fmb = fm[:].unsqueeze(2).broadcast_to([128, 25, 3])
            nc.vector.tensor_tensor(msk[:], normT[:, :, 1:4], fmb, op=ALU.is_ge)
            nc.vector.tensor_tensor(
                c5v[:, :, 1:4], normT[:, :, 1:4], msk[:], op=ALU.mult
            )
            # h-pad rows of the last block must contribute nothing
            nc.vector.memset(c5v[64:128, 24, 0:5], 0.0)

        # ============ phase C: q, k ============
        with tc.tile_pool(name="pCp", bufs=4, space=bass.MemorySpace.PSUM) as pCp:
            MCH = [(qA, 0, 128), (qB, 128, 64), (kA, 192, 128), (kB, 320, 64)]
            for no, nl in NCHP:
                for mi, (dst, mo, ml) in enumerate(MCH):
                    mlp = 128 if ml == 128 else 65  # pad 64 -> 65 stationary
                    qp = pCp.tile([128, 512], F32, tag="qkpsum")
                    nc.tensor.matmul(
                        qp[0:mlp, 0:nl], qkA[:, mo:mo + mlp], f_a[:, no:no + nl],
                        start=True, stop=False,
                    )
                    nc.tensor.matmul(
                        qp[0:mlp, 0:nl], qkB[:, mo:mo + mlp], f_b[:, no:no + nl],
                        start=False, stop=True,
                    )
                    if mi % 2 == 0:
                        nc.vector.tensor_copy(dst[0:ml, no:no + nl], qp[0:ml, 0:nl])
                    else:
                        nc.scalar.copy(dst[0:ml, no:no + nl], qp[0:ml, 0:nl])

        # ============ phase D: attention ============
        with tc.tile_pool(name="pDe", bufs=3) as pDe, \
             tc.tile_pool(name="pDr", bufs=2) as pDr, \
             tc.tile_pool(name="pDs", bufs=2, space=bass.MemorySpace.PSUM) as pDs, \
             tc.tile_pool(name="pDo", bufs=2, space=bass.MemorySpace.PSUM) as pDo:
            for ko, kl in KSUP:
                nkb = (kl + 511) // 512
                pout = pDo.tile([65, 1024], F32, tag="pout")
                for bi in range(25):
                    ho = 128 * bi
                    sp = pDs.tile([128, 1024], F32, tag="spsum")
                    for kb in range(nkb):
                        kbl = min(512, kl - kb * 512)
                        nc.tensor.matmul(
                            sp[:, kb * 512:kb * 512 + kbl], qA[:, ho:ho + 128],
                            kA[:, ko + kb * 512:ko + kb * 512 + kbl],
                            start=True, stop=False,
                        )
                    for kb in range(nkb):
                        kbl = min(512, kl - kb * 512)
                        nc.tensor.matmul(
                            sp[:, kb * 512:kb * 512 + kbl], qB[:, ho:ho + 128],
                            kB[:, ko + kb * 512:ko + kb * 512 + kbl],
                            start=False, stop=True,
                        )
                    et = pDe.tile([128, 1024], BF16, tag="exptile")
                    nc.scalar.activation(et[:, 0:kl], sp[:, 0:kl], AF.Exp)
                    for kb in range(nkb):
                        kbl = min(512, kl - kb * 512)
                        nc.tensor.matmul(
                            pout[:, kb * 512:kb * 512 + kbl],
                            camT[:, bi, :],
                            et[:, kb * 512:kb * 512 + kbl],
                            start=(bi == 0), stop=(bi == 24),
                        )
                ot5 = pDr.tile([5, 1024], F32, tag="ot5")
                nc.vector.tensor_copy(ot5[:, 0:kl], pout[0:5, 0:kl])
                den = pDr.tile([1, 1024], F32, tag="den")
                nc.sync.dma_start(den[0:1, 0:kl], ot5[4:5, 0:kl])
                rcp = pDr.tile([1, 1024], F32, tag="rcp")
                rsc = pDr.tile([1, 1024], F32, tag="rsc")
                nc.vector.reciprocal_approx_accurate(
                    rcp[0:1, 0:kl], den[0:1, 0:kl], rsc[0:1, 0:kl]
                )
                rb = pDr.tile([4, 1024], F32, tag="rb")
                nc.gpsimd.partition_broadcast(rb[:, 0:kl], rcp[0:1, 0:kl])
                nc.gpsimd.tensor_tensor(
                    out_sb[:, ko:ko + kl], ot5[0:4, 0:kl], rb[:, 0:kl],
                    op=ALU.mult,
                )
                nc.sync.dma_start(
                    d_out.ap()[:, ko:ko + kl], out_sb[:, ko:ko + kl]
                )

    nc.compile()
    return nc


def _get_program():
    if "nc" not in _CACHE:
        _CACHE["nc"] = _build_program()
    return _CACHE["nc"]


def _host_prep(inputs: dict) -> list[dict]:
    import ml_dtypes

    BFNP = ml_dtypes.bfloat16

    x = np.asarray(inputs["x"], np.float32)
    x2 = np.asarray(inputs["x2"], np.float32)
    deep3 = np.asarray(inputs["deep3"], np.float32)
    _4 = np.asarray(inputs["_4"], np.float32)
    fc8_w = np.asarray(inputs["fc8_w"], np.float32)
    f83_w = np.asarray(inputs["f83_w"], np.float32)
    f84_w = np.asarray(inputs["f84_w"], np.float32)
    f91_w = np.asarray(inputs["f91_w"], np.float32)
    f92_w = np.asarray(inputs["f92_w"], np.float32)

    n = x.shape[0]
    fc8T = np.zeros((512, 65), np.float32)
    fc8T[:, 0:4] = fc8_w.T
    f83T = np.ascontiguousarray(f83_w.T.astype(BFNP))    # [128, 64]
    f84T = np.zeros((384, 128), np.float32)
    f84T[0:320] = f84_w.T
    f84T = f84T.astype(BFNP)
    # f channel permutation: [f8_4 (128), f8_3 (64), x_s (3)]
    perm = np.concatenate([np.arange(67, 195), np.arange(3, 67), np.arange(3)])
    wqk = np.concatenate([f91_w, f92_w], axis=0)[:, perm]  # [384, 195]
    wqkT = np.ascontiguousarray(wqk.T)  # [195, 384]
    qkA = np.zeros((128, 385), np.float32)
    qkA[:, 0:384] = wqkT[0:128]
    qkB = np.zeros((67, 385), np.float32)
    qkB[:, 0:384] = wqkT[128:195]
    qkA = qkA.astype(BFNP)
    qkB = qkB.astype(BFNP)
    cres = np.ascontiguousarray(
        _combined_resize_coeff().reshape(128, 56 * 112).astype(BFNP)
    )
    rh448 = _resize_mat(448, 56).astype(BFNP)
    rw448 = rh448  # same matrix for H and W (448x448 -> 56x56)

    shared = {
        "fc8T": fc8T, "f83T": f83T, "f84T": f84T, "qkA": qkA, "qkB": qkB,
        "cres": cres, "rh448": rh448, "rw448": rw448,
    }
    in_maps = []
    for i in range(n):
        m = dict(shared)
        x4i = _4[i].reshape(4, 128, HW).transpose(1, 0, 2)
        m["x4"] = np.ascontiguousarray(x4i)           # [128, 4, 3136] f32
        d3 = np.zeros((384, HW), np.float32)
        d3[0:320] = deep3[i].reshape(320, HW)
        m["deep3"] = d3.astype(BFNP)
        x2i = x2[i].reshape(128, 112 * 112).astype(BFNP)
        m["x2a"] = np.ascontiguousarray(x2i[:, 0:6272])
        m["x2b"] = np.ascontiguousarray(x2i[:, 6272:12544])
        m["x"] = np.ascontiguousarray(x[i].transpose(1, 0, 2).astype(BFNP))
        in_maps.append(m)
    return in_maps


def _install_ntff_hook() -> bool:
    """Register the NTFF profile hook that the agent image's antenv lacks."""
    try:
        import types

        import antenv

        if "antenv.axon_hooks" not in sys.modules:
            mod = types.ModuleType("antenv.axon_hooks")
            store = {"h": None}
            mod.set_axon_ntff_profile_hook = lambda h: store.update(h=h)
            mod.get_axon_ntff_profile_hook = lambda: store["h"]
            sys.modules["antenv.axon_hooks"] = mod
            antenv.axon_hooks = mod
            from trn_agent_boot.trn_boot import _ntff_profile_via_ctypes

            hook = _ntff_profile_via_ctypes("/opt/axon/libaxon_pjrt.so")
            if hook is None:
                return False
            mod.set_axon_ntff_profile_hook(hook)
        return sys.modules["antenv.axon_hooks"].get_axon_ntff_profile_hook() is not None
    except Exception as e:  # profiling is best-effort
        print(f"ntff hook install failed: {e}", file=sys.stderr)
        return False


def kernel(**inputs) -> np.ndarray:
    nc = _get_program()
    in_maps = _host_prep(inputs)
    trace = bool(int(os.environ.get("KERNEL_PROFILE", "0")))
    if trace:
        trace = _install_ntff_hook()
    res = run_bass_kernel_spmd(nc, in_maps, core_ids=list(range(N_CORES)),
                               trace=trace)
    _CACHE["last_result"] = res
    out = np.stack([r["out"] for r in res.results]).reshape(8, 4, 56, 56)
    return out.astype(np.float32)
